# revision 30
# baseline (speedup 1.0000x reference)
"""Trainium2 Bass kernel for 2-layer LSTM (H=32, in=1) + MLP head.

Problem: x [4096, 512, 1] f32 -> y [4096, 1] f32.
Strategy: pure data parallel over 8 cores (512 batch each).

Per-core layout (block-diagonal, batch-on-partition):
  batch b = g*128 + j  (g in 0..3 groups, j in 0..127)
  partition dim = j always; free dim = (group, gate/hidden)

Per tick n (0..512): layer-0 step n and layer-1 step n-1, explicitly
software-pipelined so the two recurrence chains interleave on every
engine (L1's tail is flushed at the top of the next iteration).
  Gate order: torch [i, f, g, o]; the g-gate's weights/biases are
  pre-scaled by 2 so one Sigmoid covers i,f,g (tanh(z) = 2*sigmoid(2z)-1,
  fixed up by a DVE TensorScalar which gets the 4x perf mode). The o-gate
  gets its own small Sigmoid into a separate tile (off the critical path)
  and is transposed early (PE) so the recurrence tail is:
    tanh(c) -> PE-transpose(th) -> hT = sigmoidO_T * thT (one DVE op
  reading the transpose straight from PSUM, fusing copy-back and multiply).
  Cell state c is kept in fp16 so the c-update TensorTensors get the DVE
  2x mode; per-core steady-state cycle is ~3.03us, bounded by the serial
  chain mm -> sigmoid -> c-update -> tanh -> transpose -> h-mult -> mm
  with each hop paying the producer's write-ack latency.
"""

import os
import numpy as np
import ml_dtypes
from contextlib import ExitStack

import concourse.bass as bass
import concourse.tile as tile
import concourse.bacc as bacc
from concourse import mybir
from concourse import bass_utils

F32 = mybir.dt.float32
F16 = mybir.dt.float16
BF16 = mybir.dt.bfloat16
AF = mybir.ActivationFunctionType
OP = mybir.AluOpType

H = 32
NCORES = 8
B_FULL = 4096
S_FULL = 512
BC = 512          # batch per core
NG = 4            # groups of 128 within the core batch
XCH = 64          # xs prefetch chunk (timesteps per DMA)


def _bf(x):
    return np.asarray(x, dtype=ml_dtypes.bfloat16)


def prep_shared_weights(w_ih0, w_hh0, b_ih0, b_hh0, w_ih1, w_hh1, b_ih1, b_hh1,
                        fc1_w, fc1_b, fc2_w, fc2_b):
    """Host-side preparation of the shared (replicated) weight tensors.

    Torch gate order [i, f, g, o] is kept; g-gate rows (64:96) are scaled
    by 2 so a single Sigmoid covers i,f,g (tanh(z) = 2*sigmoid(2z) - 1)."""
    gscale = np.ones((128, 1), np.float32)
    gscale[64:96] = 2.0

    def reord(w):  # [128, k] -> g-gate rows scaled x2
        return w.astype(np.float32) * gscale

    w_hh0p = reord(w_hh0)            # [128, 32]
    w_ih0p = reord(w_ih0)            # [128, 1]
    b0p = (b_ih0 + b_hh0).astype(np.float32) * gscale[:, 0]
    w_hh1p = reord(w_hh1)
    w_ih1p = reord(w_ih1)
    b1p = (b_ih1 + b_hh1).astype(np.float32) * gscale[:, 0]

    def blockdiag(wp):  # wp [128 gates, K] -> [NG*K, NG*128]
        k = wp.shape[1]
        out = np.zeros((NG * k, NG * 128), np.float32)
        for g in range(NG):
            out[g * k:(g + 1) * k, g * 128:(g + 1) * 128] = wp.T
        return out

    wbd_hh0 = blockdiag(w_hh0p)      # [128, 512]
    wbd_hh1 = blockdiag(w_hh1p)      # [128, 512]
    wbd_ih1 = blockdiag(w_ih1p)      # [128, 512]
    # const rhs for layer0: rows 0..3 = x block-diag w_ih0 row, row 4 = bias0
    rc0 = np.zeros((5, NG * 128), np.float32)
    for g in range(NG):
        rc0[g, g * 128:(g + 1) * 128] = w_ih0p[:, 0]
    rc0[4] = np.tile(b0p, NG)
    rc1 = np.tile(b1p, NG)[None, :]  # [1, 512]

    return {
        "wbd_hh0": _bf(wbd_hh0),
        "wbd_hh1": _bf(wbd_hh1),
        "wbd_ih1": _bf(wbd_ih1),
        "rc0": _bf(rc0),
        "rc1": _bf(rc1),
        "id128": _bf(np.eye(128, dtype=np.float32)),
        "fc1T": _bf(fc1_w.T.copy()),               # [32, 16]
        "fc1b": fc1_b.reshape(16, 1).astype(np.float32),
        "fc2T": _bf(fc2_w.T.copy()),               # [16, 1]
        "fc2b": fc2_b.reshape(1, 1).astype(np.float32),
    }


def prep_core_x(x_core, s_len):
    """x_core [512, s_len] f32 -> xs [5, s_len*128] bf16 (rows 0..3 x per
    group time-major, row 4 ones)."""
    xs = np.ones((5, s_len * 128), np.float32)
    xr = x_core.reshape(NG, 128, s_len)             # [g, j, t]
    xs[:4] = xr.transpose(0, 2, 1).reshape(NG, s_len * 128)  # [g, t*128+j]
    return _bf(xs)


def build_program(s_len, num_devices=NCORES):
    nc = bacc.Bacc("TRN2", target_bir_lowering=False, debug=False,
                   enable_asserts=False, num_devices=num_devices)
    d = {}
    def din(name, shape, dt):
        d[name] = nc.dram_tensor(name, shape, dt, kind="ExternalInput").ap()
    din("xs", [5, s_len * 128], BF16)
    din("wbd_hh0", [128, 512], BF16)
    din("wbd_hh1", [128, 512], BF16)
    din("wbd_ih1", [128, 512], BF16)
    din("rc0", [5, 512], BF16)
    din("rc1", [1, 512], BF16)
    din("id128", [128, 128], BF16)
    din("fc1T", [32, 16], BF16)
    din("fc1b", [16, 1], F32)
    din("fc2T", [16, 1], BF16)
    din("fc2b", [1, 1], F32)
    y = nc.dram_tensor("y", [BC, 1], F32, kind="ExternalOutput").ap()

    with tile.TileContext(nc) as tc:
        with ExitStack() as ctx:
            singles = ctx.enter_context(tc.tile_pool(name="singles", bufs=1))
            psmm = ctx.enter_context(tc.tile_pool(name="psmm", bufs=2, space="PSUM"))
            pstr = ctx.enter_context(tc.tile_pool(name="pstr", bufs=1, space="PSUM"))
            sigp = ctx.enter_context(tc.tile_pool(name="sigp", bufs=6))
            smallp = ctx.enter_context(tc.tile_pool(name="smallp", bufs=6))
            xsp = ctx.enter_context(tc.tile_pool(name="xsp", bufs=2))

            # ---- load constants ----
            def load(name, shape, dt):
                t = singles.tile(shape, dt, tag=name)
                nc.sync.dma_start(t[:], d[name][:, :])
                return t
            wbd_hh0 = load("wbd_hh0", [128, 512], BF16)
            wbd_hh1 = load("wbd_hh1", [128, 512], BF16)
            wbd_ih1 = load("wbd_ih1", [128, 512], BF16)
            rc0 = load("rc0", [5, 512], BF16)
            rc1 = load("rc1", [1, 512], BF16)
            id128 = load("id128", [128, 128], BF16)
            fc1T = load("fc1T", [32, 16], BF16)
            fc1b = load("fc1b", [16, 1], F32)
            fc2T = load("fc2T", [16, 1], BF16)
            fc2b = load("fc2b", [1, 1], F32)

            ones1 = singles.tile([1, 128], BF16)
            nc.vector.memset(ones1[:], 1.0)

            # persistent state
            hT0 = singles.tile([128, 128], BF16)
            hT1 = singles.tile([128, 128], BF16)
            c0 = singles.tile([128, NG, H], F16)
            c1 = singles.tile([128, NG, H], F16)
            nc.vector.memset(hT0[:], 0.0)
            nc.vector.memset(hT1[:], 0.0)
            nc.vector.memset(c0[:], 0.0)
            nc.vector.memset(c1[:], 0.0)

            n_chunks = (s_len + XCH - 1) // XCH
            xs_chunks = [None] * n_chunks

            def get_xs(n):
                ch = n // XCH
                if xs_chunks[ch] is None:
                    t = xsp.tile([5, XCH * 128], BF16, tag=f"xs{ch % 2}")
                    lo = ch * XCH * 128
                    hi = min((ch + 1) * XCH, s_len) * 128
                    nc.sync.dma_start(t[:, 0:hi - lo], d["xs"][:, lo:hi])
                    xs_chunks[ch] = t
                off = (n % XCH) * 128
                return xs_chunks[ch][:, off:off + 128]

            get_xs(0)

            def sigma_main(l, ps_l):
                """sigmoid over [i, f, g2] (96 gates per group)."""
                sig = sigp.tile([128, NG, 96], BF16, tag=f"sig{l}", name=f"sig{l}")
                nc.scalar.activation(sig[:], ps_l[:, :, 0:96], AF.Sigmoid)
                return sig

            def sigma_o(l, ps_l):
                """o-gate sigmoid + early PE transpose of it."""
                sigo = sigp.tile([128, NG, H], BF16, tag=f"sigo{l}", name=f"sigo{l}")
                nc.scalar.activation(sigo[:], ps_l[:, :, 96:128], AF.Sigmoid)
                ptO = pstr.tile([128, 128], BF16, tag=f"ptO{l}", name=f"ptO{l}")
                nc.tensor.transpose(ptO[:], sigo[:], id128[:])
                return ptO

            def c_stage(l, sig, ptO, c_l):
                """DVE c update + copy of transposed o-gate."""
                g2 = smallp.tile([128, NG, H], BF16, tag=f"g2{l}", name=f"g2{l}")
                nc.vector.tensor_scalar(g2[:], sig[:, :, 64:96], 2.0, -1.0,
                                        OP.mult, OP.add)
                nc.vector.tensor_tensor(c_l[:], sig[:, :, 32:64], c_l[:], OP.mult)
                u = smallp.tile([128, NG, H], BF16, tag=f"u{l}", name=f"u{l}")
                nc.vector.tensor_tensor(u[:], sig[:, :, 0:32], g2[:], OP.mult)
                nc.vector.tensor_tensor(c_l[:], c_l[:], u[:], OP.add)
                soT = smallp.tile([128, 128], BF16, tag=f"soT{l}", name=f"soT{l}")
                nc.vector.tensor_copy(soT[:], ptO[:])
                return soT

            def tanh_stage(l, c_l):
                th = smallp.tile([128, NG, H], BF16, tag=f"th{l}", name=f"th{l}")
                nc.scalar.activation(th[:], c_l[:], AF.Tanh)
                return th

            def tail_transpose(l, th):
                ptT = pstr.tile([128, 128], BF16, tag=f"ptT{l}", name=f"ptT{l}")
                nc.tensor.transpose(ptT[:], th[:], id128[:])
                return ptT

            def tail_mult(l, ptT, soT, hT_l):
                nc.vector.tensor_tensor(hT_l[:], soT[:], ptT[:], OP.mult)

            def tail_stage(l, th, soT, hT_l):
                """PE transpose of tanh(c) + fused copy-back/multiply -> hT."""
                tail_mult(l, tail_transpose(l, th), soT, hT_l)

            pend1 = None   # (th1, soT1) of L1 step n-2, tail flushed next iter
            for n in range(s_len + 1):
                do0 = n < s_len
                do1 = n >= 1
                if do0 and (n % XCH) == 0 and n + XCH < s_len:
                    get_xs(n + XCH)
                # --- PE: critical L0 matmul first
                if do0:
                    ps0 = psmm.tile([128, NG, 128], F32, tag="ps0", name="ps0")
                    xs_t = get_xs(n)
                    nc.tensor.matmul(ps0[:], xs_t, rc0[:], start=True, stop=False)
                    nc.tensor.matmul(ps0[:], hT0[:], wbd_hh0[:], start=False, stop=True)
                # --- flush L1 tail from previous iteration (updates hT1);
                #     PE transpose early, DVE multiply after L0's c-chain
                ptT1p = None
                if pend1 is not None:
                    ptT1p = tail_transpose(1, pend1[0])
                # --- L1 matmuls (read the soon-updated hT1 and current hT0)
                if do1:
                    ps1 = psmm.tile([128, NG, 128], F32, tag="ps1", name="ps1")
                    nc.tensor.matmul(ps1[:], ones1[:], rc1[:], start=True, stop=False)
                    nc.tensor.matmul(ps1[:], hT0[:], wbd_ih1[:], start=False, stop=False)
                # --- interleaved stages: L0 sigma, L0 c, L1 sigma, L0 tanh,
                #     L1 c, L0 tail, L1 tanh (L1 tail next iter)
                if do0:
                    sig0 = sigma_main(0, ps0)
                    ptO0 = sigma_o(0, ps0)
                    soT0 = c_stage(0, sig0, ptO0, c0)
                if ptT1p is not None:
                    tail_mult(1, ptT1p, pend1[1], hT1)
                    pend1 = None
                if do1:
                    nc.tensor.matmul(ps1[:], hT1[:], wbd_hh1[:], start=False, stop=True)
                if do1:
                    sig1 = sigma_main(1, ps1)
                if do0:
                    th0 = tanh_stage(0, c0)
                if do1:
                    ptO1 = sigma_o(1, ps1)
                    soT1 = c_stage(1, sig1, ptO1, c1)
                if do0:
                    tail_stage(0, th0, soT0, hT0)
                if do1:
                    th1 = tanh_stage(1, c1)
                    pend1 = (th1, soT1)
            if pend1 is not None:
                tail_stage(1, *pend1, hT1)
                pend1 = None

            # ---- MLP head on h1_last (= hT1) ----
            hstack = smallp.tile([32, 512], BF16, tag="hstack")
            for g in range(NG):
                nc.sync.dma_start(hstack[0:32, g * 128:(g + 1) * 128],
                                  hT1[32 * g:32 * (g + 1), :])
            pm1 = pstr.tile([16, 512], F32, tag="ptO0")
            nc.tensor.matmul(pm1[:], fc1T[:], hstack[:], start=True, stop=True)
            z1 = smallp.tile([16, 512], F32, tag="z1")
            nc.scalar.activation(z1[:], pm1[:], AF.Identity, bias=fc1b[:])
            a1 = smallp.tile([16, 512], BF16, tag="a1")
            nc.vector.tensor_scalar(a1[:], z1[:], 0.2, None, OP.mult)
            nc.vector.tensor_tensor(a1[:], z1[:], a1[:], OP.max)
            pm2 = pstr.tile([1, 512], F32, tag="ptT0")
            nc.tensor.matmul(pm2[:], fc2T[:], a1[:], start=True, stop=True)
            ysb = smallp.tile([1, 512], F32, tag="ysb")
            nc.scalar.activation(ysb[:], pm2[:], AF.Identity, bias=fc2b[:])
            nc.sync.dma_start(y[:, :], ysb[:])

    nc.compile()
    return nc


_CACHE = {}


def _get_program():
    if "nc" not in _CACHE:
        _CACHE["nc"] = build_program(S_FULL)
    return _CACHE["nc"]


def kernel(x, w_ih0, w_hh0, b_ih0, b_hh0, w_ih1, w_hh1, b_ih1, b_hh1,
           fc1_w, fc1_b, fc2_w, fc2_b):
    x = np.asarray(x, np.float32)
    shared = prep_shared_weights(
        np.asarray(w_ih0, np.float32), np.asarray(w_hh0, np.float32),
        np.asarray(b_ih0, np.float32), np.asarray(b_hh0, np.float32),
        np.asarray(w_ih1, np.float32), np.asarray(w_hh1, np.float32),
        np.asarray(b_ih1, np.float32), np.asarray(b_hh1, np.float32),
        np.asarray(fc1_w, np.float32), np.asarray(fc1_b, np.float32),
        np.asarray(fc2_w, np.float32), np.asarray(fc2_b, np.float32))
    nc = _get_program()
    in_maps = []
    for c in range(NCORES):
        xc = x[c * BC:(c + 1) * BC, :, 0]          # [512, 512]
        m = dict(shared)
        m["xs"] = prep_core_x(xc, S_FULL)
        in_maps.append(m)
    res = bass_utils.run_bass_kernel_spmd(
        nc, in_maps, core_ids=list(range(NCORES)),
        trace=bool(int(os.environ.get("KERNEL_TRACE", "0"))))
    _CACHE["last_results"] = res
    y = np.concatenate([res.results[c]["y"] for c in range(NCORES)], axis=0)
    return y.astype(np.float32)
